# revision 6
# baseline (speedup 1.0000x reference)
"""Two-layer GCN (nn_Method_GCN_11098195493080) as a Bass/Tile kernel on 8
Trainium2 NeuronCores.

v2 strategy (1D graph partition, dst-owned edges, halo via AllGather):
  - Nodes sharded 8 ways; per core, slots are tile-major (slot = t*128+p)
    and split into 4 row-slices of nt/4 tiles each.  The halo exchange is
    4 *pipelined* per-slice AllGathers per layer, fired as soon as each
    slice's table rows are written - they overlap compute instead of
    serializing the kernel.
  - Layer 1: y = dinv*(x@W1) on PE (bf16), row-major per tile.
  - Aggregation (both layers): batched dma_gather (SWDGE, 4 queues)
    fetches 256B bf16 rows edge-major.  The segment-sum runs on PE in
    *flipped* orientation: out_T[hid,dst] += msg_block.T @ onehot with
    the gathered 64-col block as the (cheap) stationary operand and a
    [128dst x nb]-laid-out one-hot streamed at N=128.  One-hots are
    built on DVE with a 2x-eligible bf16 is_equal (iota_rep == dstl).
  - Self loops are plain edges in the gather lists (table rows already
    carry dinv_src, so norm = dinv_src*dinv_dst falls out uniformly).
  - Layer-2 epilogue: the column-major accumulator is exactly the lhsT
    of the @W2 matmul - no transposes.  log_softmax is deferred
    (running -max and sum-exp per tile, one fixup pass at the end).
  - Host-side work is integer graph partitioning only: slice-local
    block-packing balancer (targets 4 gather blocks per (tile,chunk)),
    per-group index sort for HBM locality.
"""

import numpy as np
import ml_dtypes

import concourse.bass as bass
import concourse.bacc as bacc
import concourse.mybir as mybir
import concourse.tile as tile
from concourse import bass_utils
from concourse.masks import make_identity

F32 = mybir.dt.float32
BF16 = mybir.dt.bfloat16
I16 = mybir.dt.int16
AF = mybir.ActivationFunctionType
OP = mybir.AluOpType
NPBF16 = ml_dtypes.bfloat16

N_CORES = 8
N_SLICES = 4          # row-slices per core = AllGather pipeline stages
P = 128               # partitions / dst-tile size
PAD_DST = 254.0       # dst_local value whose one-hot column is all-zero
TROW = 128            # table row width (bf16): hid data + pad = 256B
CAPC = 30             # max gather blocks per (supertile, chunk)
MAX_ST_TILES = 6      # max tiles per supertile (psum bank bound)


def _balance_slice(d4, ntile, cap_nodes=P):
    """Assign nodes of one (core, slice) to ntile tiles, minimizing the
    number of 128-slot gather blocks per (tile, chunk).  d4 = per-chunk
    in-degree [n, 4].  Returns slot index (t*128+p, local to slice)."""
    n = d4.shape[0]
    order = np.argsort(-d4.sum(axis=1), kind="stable")
    loads = np.zeros((ntile, N_SLICES), dtype=np.int64)
    counts = np.zeros(ntile, dtype=np.int64)
    slot_of = np.empty(n, dtype=np.int64)
    blocks = np.zeros((ntile, N_SLICES), dtype=np.int64)  # ceil(loads/128)
    full = np.zeros(ntile, dtype=np.int64)
    for node in order:
        d = d4[node]
        nl = loads + d
        nb = (nl + P - 1) // P
        # primary: new blocks started; secondary: total load (LPT); prefer
        # low tile index so overflow blocks cluster on the same tiles
        # across all cores.
        score = (nb - blocks).sum(axis=1) * (1 << 30) + nl.max(axis=1) + full
        t = int(np.argmin(score))
        loads[t] += d
        blocks[t] = (loads[t] + P - 1) // P
        slot_of[node] = t * cap_nodes + counts[t]
        counts[t] += 1
        if counts[t] >= cap_nodes:
            full[t] = 1 << 40
    return slot_of


class Plan:
    """Static, core-uniform schedule derived from the (integer) graph."""

    def __init__(self, n_nodes, fin, hid, fout, edge_index, n_cores=N_CORES):
        assert n_nodes % n_cores == 0
        self.n_nodes = n_nodes
        self.fin, self.hid, self.fout = fin, hid, fout
        self.n_cores = n_cores
        self.base = n_nodes // n_cores          # real nodes per core
        # tiles: multiple of N_SLICES, some slack for balancing
        tps = (self.base // N_SLICES + P - 1) // P + 1   # tiles per slice
        self.tps = tps
        self.nt = tps * N_SLICES
        self.nsh = self.nt * P
        self.slice_sz = tps * P                  # rows per slice
        self.ch = self.slice_sz * n_cores        # rows per gather chunk
        assert self.ch <= 32767, "chunk must fit int16 gather index"
        self.ntab = self.ch * N_SLICES
        assert fin % P == 0
        self.kch = fin // P
        self.nps = self.base // N_SLICES         # real nodes per slice

        # --- self loops as ordinary edges -----------------------------
        loop = np.arange(n_nodes, dtype=np.int64)
        src = np.concatenate([np.asarray(edge_index[0], dtype=np.int64), loop])
        dst = np.concatenate([np.asarray(edge_index[1], dtype=np.int64), loop])
        s_owner = src // self.base
        d_owner = dst // self.base
        s_local = src - s_owner * self.base
        d_local = dst - d_owner * self.base
        # node -> slice by local id (deterministic, pre-balance)
        s_slice = s_local // self.nps
        np.minimum(s_slice, N_SLICES - 1, out=s_slice)
        d_slice = d_local // self.nps
        np.minimum(d_slice, N_SLICES - 1, out=d_slice)

        # --- per (core, slice) balance on per-chunk in-degree ---------
        self.slot_of = np.empty((n_cores, self.base), dtype=np.int64)
        for c in range(n_cores):
            for j in range(N_SLICES):
                lo, hi = j * self.nps, min((j + 1) * self.nps, self.base)
                sel = (d_owner == c) & (d_slice == j)
                dl = d_local[sel] - lo
                ck = s_slice[sel]
                d4 = np.zeros((hi - lo, N_SLICES), dtype=np.int64)
                np.add.at(d4, (dl, ck), 1)
                sl = _balance_slice(d4, tps)
                self.slot_of[c, lo:hi] = j * self.slice_sz + sl

        # --- slot-space edge endpoints --------------------------------
        d_slot = self.slot_of[d_owner, d_local]
        s_slot = self.slot_of[s_owner, s_local]
        chunk = s_slot // self.slice_sz            # = s_slice (consistent)
        idx_local = s_owner * self.slice_sz + (s_slot - chunk * self.slice_sz)
        tile_id = d_slot // P
        dloc = (d_slot % P).astype(np.float32)

        key = (d_owner * self.nt + tile_id) * N_SLICES + chunk
        # sort by group; within group ascending table index (HBM locality)
        order = np.lexsort((idx_local, key))
        self._sorted_idx = idx_local[order]
        self._sorted_dloc = dloc[order]
        ngroups = n_cores * self.nt * N_SLICES
        sizes = np.bincount(key, minlength=ngroups).reshape(
            n_cores, self.nt, N_SLICES
        )
        self._gstart = np.zeros(ngroups + 1, dtype=np.int64)
        np.cumsum(sizes.reshape(-1), out=self._gstart[1:])
        self.sizes = sizes

        nb = (sizes.max(axis=0) + P - 1) // P             # [nt, N_SLICES]
        nb[:, 0] = np.maximum(nb[:, 0], 1)                # >=1 block per tile
        self.nb = nb
        self.bp = np.zeros((N_SLICES, self.nt + 1), dtype=np.int64)
        np.cumsum(nb.T, axis=1, out=self.bp[:, 1:])
        self.blocks_c = self.bp[:, -1].copy()
        self.slots_c = self.blocks_c * P
        self.total_slots = int(self.slots_c.sum())

        # --- supertiles: within slices, capped blocks per chunk -------
        self.supertiles = []
        for j in range(N_SLICES):
            t0 = j * tps
            tend = (j + 1) * tps
            while t0 < tend:
                t1 = t0 + 1
                while t1 < tend and t1 - t0 < MAX_ST_TILES:
                    if any(
                        int(nb[t0:t1 + 1, c].sum()) > CAPC
                        for c in range(N_SLICES)
                    ):
                        break
                    t1 += 1
                self.supertiles.append((t0, t1))
                t0 = t1
        self.max_sg_blocks = max(
            max(int(self.nb[a:b, c].sum()) for c in range(N_SLICES))
            for a, b in self.supertiles
        )
        assert self.max_sg_blocks <= CAPC

        # --- degrees (incl self loop), per core [P, nt] slot order ----
        deg = np.bincount(dst, minlength=n_nodes).astype(np.float32)
        self.degw = np.ones((n_cores, P, self.nt), dtype=np.float32)
        self.dinvrow = np.ones((n_cores, 1, self.nsh), dtype=np.float32)
        for c in range(n_cores):
            d = np.ones(self.nsh, dtype=np.float32)
            d[self.slot_of[c]] = deg[c * self.base:(c + 1) * self.base]
            self.degw[c] = d.reshape(self.nt, P).T
            self.dinvrow[c, 0] = 1.0 / np.sqrt(d)

        # --- per-core gather index / dst_local arrays -----------------
        self.idx16 = []
        self.dstl = []
        for c in range(n_cores):
            idx_c, dstl_c = [], []
            for k in range(N_SLICES):
                s = int(self.slots_c[k])
                ia = np.zeros(s, dtype=np.int16)
                da = np.full(s, PAD_DST, dtype=np.float32)
                for t in range(self.nt):
                    g = (c * self.nt + t) * N_SLICES + k
                    a, b = self._gstart[g], self._gstart[g + 1]
                    o = int(self.bp[k][t]) * P
                    n = int(b - a)
                    ia[o:o + n] = self._sorted_idx[a:b].astype(np.int16)
                    da[o:o + n] = self._sorted_dloc[a:b]
                idx_c.append(np.ascontiguousarray(
                    np.tile(ia.reshape(-1, 16).T, (P // 16, 1))
                ).astype(np.int16))
                dstl_c.append(np.ascontiguousarray(
                    da.reshape(-1, P).T).astype(NPBF16))
            self.idx16.append(idx_c)
            self.dstl.append(dstl_c)


def build_nc(plan: Plan):
    nc = bacc.Bacc(
        "TRN2",
        target_bir_lowering=False,
        debug=False,
        enable_asserts=False,
        num_devices=plan.n_cores,
        num_swdge_queues=N_SLICES,
    )
    fin, hid, fout = plan.fin, plan.hid, plan.fout
    nt, nsh, kch, tps = plan.nt, plan.nsh, plan.kch, plan.tps
    ssz = plan.slice_sz

    xT = nc.dram_tensor("xT", [fin, nsh], BF16, kind="ExternalInput")
    degw = nc.dram_tensor("degw", [P, nt], F32, kind="ExternalInput")
    dinvr = nc.dram_tensor("dinvr", [1, nsh], F32, kind="ExternalInput")
    w1 = nc.dram_tensor("w1", [fin, hid], BF16, kind="ExternalInput")
    b1 = nc.dram_tensor("b1", [hid, 1], F32, kind="ExternalInput")
    w2 = nc.dram_tensor("w2", [hid, fout], F32, kind="ExternalInput")
    b2 = nc.dram_tensor("b2", [1, fout], F32, kind="ExternalInput")
    idx_d = [
        nc.dram_tensor(f"idx{c}", [P, int(plan.slots_c[c]) // 16], I16,
                       kind="ExternalInput")
        for c in range(N_SLICES)
    ]
    dstl_d = [
        nc.dram_tensor(f"dstl{c}", [P, int(plan.blocks_c[c])], BF16,
                       kind="ExternalInput")
        for c in range(N_SLICES)
    ]
    out = nc.dram_tensor("out", [nsh, fout], F32, kind="ExternalOutput")

    table1 = [
        nc.dram_tensor(f"table1_{c}", [plan.ch, TROW], BF16,
                       kind="Internal", addr_space="Shared")
        for c in range(N_SLICES)
    ]
    table2 = [
        nc.dram_tensor(f"table2_{c}", [plan.ch, TROW], BF16,
                       kind="Internal", addr_space="Shared")
        for c in range(N_SLICES)
    ]

    rg = [list(range(plan.n_cores))]

    with tile.TileContext(nc) as tc:
        with (
            tc.tile_pool(name="const", bufs=1) as cp,
            tc.tile_pool(name="dram", bufs=1, space="DRAM") as dp,
        ):
            # ---- constants -------------------------------------------------
            iota_rep = cp.tile([P, P, CAPC], BF16)
            nc.gpsimd.iota(iota_rep[:], pattern=[[1, P], [0, CAPC]], base=0,
                           channel_multiplier=0,
                           allow_small_or_imprecise_dtypes=True)
            identb = cp.tile([hid, hid], BF16)
            make_identity(nc, identb[:])

            w1sb = cp.tile([P, kch, hid], BF16)
            nc.sync.dma_start(
                w1sb[:], w1.ap().rearrange("(a p) f -> p a f", p=P)
            )
            w2sb = cp.tile([hid, fout], F32)
            nc.sync.dma_start(w2sb[:], w2.ap())
            b1sb = cp.tile([hid, 1], F32)
            nc.sync.dma_start(b1sb[:], b1.ap())
            b2row = cp.tile([P, fout], F32)
            nc.sync.dma_start(b2row[:], b2.ap().to_broadcast([P, fout]))

            degt = cp.tile([P, nt], F32)
            nc.sync.dma_start(degt[:], degw.ap())
            rec = cp.tile([P, nt], F32)
            nc.vector.reciprocal(rec[:], degt[:])
            dinv = cp.tile([P, nt], F32)
            nc.scalar.activation(dinv[:], rec[:], AF.Sqrt)
            dinv_col = cp.tile([hid, nsh], F32)
            nc.sync.dma_start(dinv_col[:], dinvr.ap().to_broadcast([hid, nsh]))

            idxsb = []
            dstlsb = []
            for c in range(N_SLICES):
                it = cp.tile([P, int(plan.slots_c[c]) // 16], I16,
                             tag=f"idx{c}")
                nc.sync.dma_start(it[:], idx_d[c].ap())
                idxsb.append(it)
                dt_ = cp.tile([P, int(plan.blocks_c[c])], BF16,
                              tag=f"dstl{c}")
                nc.sync.dma_start(dt_[:], dstl_d[c].ap())
                dstlsb.append(dt_)

            m_all = cp.tile([P, nt], F32)
            ssum_all = cp.tile([P, nt], F32)
            out_loc = cp.tile([P, nt, fout], F32)

            ybounce = [dp.tile([ssz, TROW], BF16, name=f"ybounce{c}")
                       for c in range(N_SLICES)]
            zbounce = [dp.tile([ssz, TROW], BF16, name=f"zbounce{c}")
                       for c in range(N_SLICES)]

            # ---- HAM warm-up: ~4us of dummy matmuls ------------------------
            with (
                tc.tile_pool(name="warm", bufs=1) as wp,
                tc.tile_pool(name="warmps", bufs=2, space="PSUM") as wpp,
            ):
                wa = wp.tile([P, P], BF16)
                nc.vector.memset(wa[:], 0.0)
                wa2 = wp.tile([P, 512], BF16)
                nc.vector.memset(wa2[:], 0.0)
                for i in range(12):
                    wps = wpp.tile([P, 512], F32, tag="wps")
                    nc.tensor.matmul(wps[:], lhsT=wa[:], rhs=wa2[:],
                                     start=True, stop=True)

            # ---- phase 1: y = dinv * (x @ W1) -> per-slice AllGather -------
            WB = 8
            with (
                tc.tile_pool(name="xload", bufs=2) as xp,
                tc.tile_pool(name="ps1", bufs=4, space="PSUM") as pp1,
                tc.tile_pool(name="ystage", bufs=2) as yp,
            ):
                xTap = xT.ap().rearrange("(a p) n -> p a n", p=P)
                y_w = None
                for wb in range(0, nt, WB):
                    nwin = min(WB, nt - wb)
                    xt = xp.tile([P, kch, P * WB], BF16, tag="xt")
                    nc.sync.dma_start(
                        xt[:, :, : P * nwin],
                        xTap[:, :, wb * P:(wb + nwin) * P],
                    )
                    for w in range(nwin):
                        t = wb + w
                        tl = t % tps
                        if tl == 0:
                            y_w = yp.tile([P, tps, hid], BF16, tag="yw")
                        ps = pp1.tile([P, hid], F32, tag="ps1")
                        for a in range(kch):
                            nc.tensor.matmul(
                                ps[:],
                                lhsT=xt[:, a, w * P:(w + 1) * P],
                                rhs=w1sb[:, a, :],
                                start=(a == 0),
                                stop=(a == kch - 1),
                            )
                        nc.vector.tensor_scalar(
                            out=y_w[:, tl, :], in0=ps[:],
                            scalar1=dinv[:, t:t + 1], scalar2=None,
                            op0=OP.mult,
                        )
                        if tl == tps - 1:
                            j = t // tps
                            nc.sync.dma_start(
                                ybounce[j][:].rearrange(
                                    "(t p) f -> p t f", p=P)[:, :, :hid],
                                y_w[:],
                            )
                            nc.gpsimd.collective_compute(
                                "AllGather", OP.bypass, replica_groups=rg,
                                ins=[ybounce[j].opt()],
                                outs=[table1[j].ap()],
                            )

            # ---- aggregation pass (both layers) ----------------------------
            def aggregate(tables, epilogue, lname):
                with (
                    tc.tile_pool(name=f"gath{lname}", bufs=3) as gp,
                    tc.tile_pool(name=f"stp{lname}", bufs=3) as stp,
                    tc.tile_pool(name=f"acc{lname}", bufs=1,
                                 space="PSUM") as pacc,
                    tc.tile_pool(name=f"eps{lname}", bufs=3) as ep,
                    tc.tile_pool(name=f"psT{lname}", bufs=2,
                                 space="PSUM") as ppT,
                ):
                    for (t0, t1) in plan.supertiles:
                        nts = t1 - t0
                        accs = [
                            pacc.tile([hid, P], F32, tag=f"acc{t - t0}",
                                      name=f"acc{t}")
                            for t in range(t0, t1)
                        ]
                        done = [0] * nts
                        total = [int(plan.nb[t].sum()) for t in range(t0, t1)]
                        for c in range(N_SLICES):
                            blk0 = int(plan.bp[c][t0])
                            blk1 = int(plan.bp[c][t1])
                            nbg = blk1 - blk0
                            if nbg == 0:
                                continue
                            yb = gp.tile([P, CAPC, TROW], BF16, tag="yb")
                            nc.gpsimd.dma_gather(
                                yb[:, :nbg, :],
                                tables[c].ap(),
                                idxsb[c][:, blk0 * 8:blk1 * 8],
                                nbg * P,
                                nbg * P,
                                TROW,
                                single_packet=False,
                                queue_num=c,
                            )
                            st = stp.tile([P, P, CAPC], BF16, tag="st")
                            nc.vector.tensor_tensor(
                                out=st[:, :, :nbg],
                                in0=iota_rep[:, :, :nbg],
                                in1=dstlsb[c][:, blk0:blk1].rearrange(
                                    "p (a b) -> p a b", a=1
                                ).to_broadcast([P, P, nbg]),
                                op=OP.is_equal,
                            )
                            for t in range(t0, t1):
                                i = t - t0
                                b0 = int(plan.bp[c][t])
                                b1_ = int(plan.bp[c][t + 1])
                                for b in range(b0, b1_):
                                    nc.tensor.matmul(
                                        accs[i][:],
                                        lhsT=yb[:, b - blk0, :hid],
                                        rhs=st[:, :, b - blk0],
                                        start=(done[i] == 0),
                                        stop=(done[i] == total[i] - 1),
                                    )
                                    done[i] += 1
                        for t in range(t0, t1):
                            epilogue(t, accs[t - t0], ep, ppT)

            # ---- layer-1 epilogue: zT = dinv*relu(dinv*acc + b1) -----------
            zstage = {}

            def epi1(t, acc, ep, ppT):
                tl = t % tps
                j = t // tps
                if tl == 0:
                    zstage[j] = ep.tile([P, tps, hid], BF16, tag="zw",
                                        name=f"zw{j}")
                u = ep.tile([hid, P], F32, tag="u")
                nc.vector.tensor_tensor(
                    out=u[:], in0=acc[:],
                    in1=dinv_col[:, t * P:(t + 1) * P], op=OP.mult,
                )
                zpre = ep.tile([hid, P], BF16, tag="zpre")
                nc.scalar.activation(zpre[:], u[:], AF.Relu,
                                     bias=b1sb[:])
                zT = ep.tile([hid, P], BF16, tag="zT")
                nc.vector.tensor_tensor(
                    out=zT[:], in0=zpre[:],
                    in1=dinv_col[:, t * P:(t + 1) * P], op=OP.mult,
                )
                pT = ppT.tile([P, hid], BF16, tag="pT")
                nc.tensor.transpose(out=pT[:], in_=zT[:], identity=identb[:])
                nc.scalar.copy(zstage[j][:, tl, :], pT[:])
                if tl == tps - 1:
                    nc.sync.dma_start(
                        zbounce[j][:].rearrange(
                            "(t p) f -> p t f", p=P)[:, :, :hid],
                        zstage[j][:],
                    )
                    nc.gpsimd.collective_compute(
                        "AllGather", OP.bypass, replica_groups=rg,
                        ins=[zbounce[j].opt()],
                        outs=[table2[j].ap()],
                    )

            aggregate(table1, epi1, "L1")

            # ---- layer-2 epilogue: log_softmax((dinv*acc2) @ W2 + b2) ------
            def epi2(t, acc, ep, ppT):
                u2 = ep.tile([hid, P], F32, tag="u2")
                nc.vector.tensor_tensor(
                    out=u2[:], in0=acc[:],
                    in1=dinv_col[:, t * P:(t + 1) * P], op=OP.mult,
                )
                po = ppT.tile([P, fout], F32, tag="po")
                nc.tensor.matmul(po[:], lhsT=u2[:], rhs=w2sb[:],
                                 start=True, stop=True)
                nc.vector.tensor_tensor(
                    out=out_loc[:, t, :], in0=po[:], in1=b2row[:], op=OP.add
                )
                nc.vector.reduce_max(m_all[:, t:t + 1], out_loc[:, t, :],
                                     axis=mybir.AxisListType.X, negate=True)
                e = ep.tile([P, fout], F32, tag="e")
                nc.scalar.activation(e[:], out_loc[:, t, :], AF.Exp,
                                     bias=m_all[:, t:t + 1],
                                     accum_out=ssum_all[:, t:t + 1])

            aggregate(table2, epi2, "L2")

            # deferred log-sum-exp: out -= log(ssum) - m_all (m_all = -max)
            lse_all = cp.tile([P, nt], F32)
            nc.scalar.activation(lse_all[:], ssum_all[:], AF.Ln)
            c_all = cp.tile([P, nt], F32)
            nc.vector.tensor_tensor(
                out=c_all[:], in0=lse_all[:], in1=m_all[:], op=OP.subtract
            )
            for t in range(nt):
                nc.vector.tensor_scalar(
                    out=out_loc[:, t, :], in0=out_loc[:, t, :],
                    scalar1=c_all[:, t:t + 1], scalar2=None,
                    op0=OP.subtract,
                )
            nc.sync.dma_start(
                out.ap().rearrange("(t p) f -> p t f", p=P), out_loc[:]
            )

    nc.compile()
    return nc


def make_in_maps(plan: Plan, x, W1, b1, W2, b2):
    x = np.asarray(x, dtype=np.float32)
    w1b = np.ascontiguousarray(W1, dtype=np.float32).astype(NPBF16)
    in_maps = []
    for c in range(plan.n_cores):
        xT = np.zeros((plan.fin, plan.nsh), dtype=NPBF16)
        xs = x[c * plan.base:(c + 1) * plan.base, :].astype(NPBF16)
        xT[:, plan.slot_of[c]] = xs.T
        m = {
            "xT": xT,
            "degw": plan.degw[c],
            "dinvr": plan.dinvrow[c],
            "w1": w1b,
            "b1": np.asarray(b1, dtype=np.float32).reshape(-1, 1),
            "w2": np.ascontiguousarray(W2, dtype=np.float32),
            "b2": np.asarray(b2, dtype=np.float32).reshape(1, -1),
        }
        for k in range(N_SLICES):
            m[f"idx{k}"] = plan.idx16[c][k]
            m[f"dstl{k}"] = plan.dstl[c][k]
        in_maps.append(m)
    return in_maps


_CACHE = {}


def _get_compiled(n_nodes, fin, hid, fout, edge_key, edge_index):
    key = (n_nodes, fin, hid, fout, edge_key)
    if key not in _CACHE:
        plan = Plan(n_nodes, fin, hid, fout, edge_index)
        nc = build_nc(plan)
        _CACHE[key] = (plan, nc)
    return _CACHE[key]


def kernel(x, edge_index, W1, b1, W2, b2, _trace=False):
    x = np.asarray(x)
    edge_index = np.asarray(edge_index)
    n_nodes, fin = x.shape
    hid = np.asarray(W1).shape[1]
    fout = np.asarray(W2).shape[1]
    edge_key = hash(edge_index.tobytes())
    plan, nc = _get_compiled(n_nodes, fin, hid, fout, edge_key, edge_index)
    in_maps = make_in_maps(plan, x, W1, b1, W2, b2)
    res = bass_utils.run_bass_kernel_spmd(
        nc, in_maps, core_ids=list(range(plan.n_cores)), trace=_trace
    )
    parts = [
        res.results[c]["out"][plan.slot_of[c], :]
        for c in range(plan.n_cores)
    ]
    out = np.concatenate(parts, axis=0).astype(np.float32)
    kernel.last_results = res
    return out


# revision 11
# speedup vs baseline: 1.2574x; 1.2574x over previous
"""Two-layer GCN (nn_Method_GCN_11098195493080) as a Bass/Tile kernel on 8
Trainium2 NeuronCores.

v3 strategy (1D graph partition, dst-owned edges, halo via AllGather):
  - Nodes sharded 8 ways; per core, slots are tile-major (slot = t*128+p)
    and split into 4 row-slices of nt/4 tiles each.  The halo exchange is
    4 *pipelined* per-slice AllGathers per layer, fired as soon as each
    slice's table rows are written - they overlap compute instead of
    serializing the kernel.
  - Layer 1: y = dinv*(x@W1) on PE (bf16), row-major per tile.
  - Aggregation (both layers): batched dma_gather (SWDGE, 4 queues)
    fetches 256B bf16 rows edge-major; a one-hot ST[128 edges, 128 dst]
    turns the segment sum into PE matmuls accumulated in PSUM (one
    psum tile per supertile, column-sliced per dst tile).  The one-hots
    are built on DVE at 2x rate: dstl is stored duplicated in adjacent
    pairs so the comparison's broadcast operand still ends in a real
    stride-1 pair (keeps the DVE 16-bit 2-ops/cycle mode eligible)
    while the output ST stays block-contiguous for LDWEIGHTS.
  - Self loops are plain edges in the gather lists (table rows already
    carry dinv_src, so norm = dinv_src*dinv_dst falls out uniformly).
  - Gather indices are sorted ascending within each (tile, chunk) group
    for HBM locality.  A short dummy-matmul burst at kernel start warms
    the PE HAM clock gate.
  - log_softmax is deferred (running -max and sum-exp per tile, one
    fixup pass at the end).
"""

import numpy as np
import ml_dtypes

import concourse.bass as bass
import concourse.bacc as bacc
import concourse.mybir as mybir
import concourse.tile as tile
from concourse import bass_utils
from concourse.masks import make_identity

F32 = mybir.dt.float32
BF16 = mybir.dt.bfloat16
I16 = mybir.dt.int16
AF = mybir.ActivationFunctionType
OP = mybir.AluOpType
NPBF16 = ml_dtypes.bfloat16

N_CORES = 8
N_SLICES = 4          # row-slices per core = AllGather pipeline stages
P = 128               # partitions / dst-tile size
PAD_DST = 254.0       # dst_local value whose one-hot row is all-zero
TROW = 128            # table row width (bf16): hid data + pad = 256B
CAPC = 30             # max gather blocks per (supertile, chunk)
MAX_ST_TILES = 6      # max tiles per supertile (one psum bank)


def _balance_slice(d4, ntile, cap_nodes=P):
    """Assign nodes of one (core, slice) to ntile tiles, minimizing the
    number of 128-slot gather blocks per (tile, chunk).  d4 = per-chunk
    in-degree [n, 4].  Returns slot index (t*128+p, local to slice)."""
    n = d4.shape[0]
    order = np.argsort(-d4.sum(axis=1), kind="stable")
    loads = np.zeros((ntile, N_SLICES), dtype=np.int64)
    counts = np.zeros(ntile, dtype=np.int64)
    slot_of = np.empty(n, dtype=np.int64)
    blocks = np.zeros((ntile, N_SLICES), dtype=np.int64)  # ceil(loads/128)
    full = np.zeros(ntile, dtype=np.int64)
    for node in order:
        d = d4[node]
        nl = loads + d
        nb = (nl + P - 1) // P
        # primary: new blocks started; secondary: total load (LPT); prefer
        # low tile index so overflow blocks cluster on the same tiles
        # across all cores.
        score = (nb - blocks).sum(axis=1) * (1 << 30) + nl.max(axis=1) + full
        t = int(np.argmin(score))
        loads[t] += d
        blocks[t] = (loads[t] + P - 1) // P
        slot_of[node] = t * cap_nodes + counts[t]
        counts[t] += 1
        if counts[t] >= cap_nodes:
            full[t] = 1 << 40
    return slot_of


class Plan:
    """Static, core-uniform schedule derived from the (integer) graph."""

    def __init__(self, n_nodes, fin, hid, fout, edge_index, n_cores=N_CORES):
        assert n_nodes % n_cores == 0
        self.n_nodes = n_nodes
        self.fin, self.hid, self.fout = fin, hid, fout
        self.n_cores = n_cores
        self.base = n_nodes // n_cores          # real nodes per core
        tps = (self.base // N_SLICES + P - 1) // P + 1   # tiles per slice
        self.tps = tps
        self.nt = tps * N_SLICES
        self.nsh = self.nt * P
        self.slice_sz = tps * P                  # rows per slice
        self.ch = self.slice_sz * n_cores        # rows per gather chunk
        assert self.ch <= 32767, "chunk must fit int16 gather index"
        self.ntab = self.ch * N_SLICES
        assert fin % P == 0
        self.kch = fin // P
        self.nps = self.base // N_SLICES         # real nodes per slice

        # --- self loops as ordinary edges -----------------------------
        loop = np.arange(n_nodes, dtype=np.int64)
        src = np.concatenate([np.asarray(edge_index[0], dtype=np.int64), loop])
        dst = np.concatenate([np.asarray(edge_index[1], dtype=np.int64), loop])
        s_owner = src // self.base
        d_owner = dst // self.base
        s_local = src - s_owner * self.base
        d_local = dst - d_owner * self.base
        # node -> slice by local id (deterministic, pre-balance)
        s_slice = np.minimum(s_local // self.nps, N_SLICES - 1)
        d_slice = np.minimum(d_local // self.nps, N_SLICES - 1)

        # --- per (core, slice) balance on per-chunk in-degree ---------
        self.slot_of = np.empty((n_cores, self.base), dtype=np.int64)
        for c in range(n_cores):
            for j in range(N_SLICES):
                lo, hi = j * self.nps, min((j + 1) * self.nps, self.base)
                sel = (d_owner == c) & (d_slice == j)
                dl = d_local[sel] - lo
                ck = s_slice[sel]
                d4 = np.zeros((hi - lo, N_SLICES), dtype=np.int64)
                np.add.at(d4, (dl, ck), 1)
                sl = _balance_slice(d4, tps)
                self.slot_of[c, lo:hi] = j * self.slice_sz + sl

        # --- slot-space edge endpoints --------------------------------
        d_slot = self.slot_of[d_owner, d_local]
        s_slot = self.slot_of[s_owner, s_local]
        chunk = s_slot // self.slice_sz            # = s_slice (consistent)
        idx_local = s_owner * self.slice_sz + (s_slot - chunk * self.slice_sz)
        tile_id = d_slot // P
        dloc = (d_slot % P).astype(np.float32)

        key = (d_owner * self.nt + tile_id) * N_SLICES + chunk
        # sort by group; keep random order within groups (ascending-sorted
        # indices make the 16 SDMA engines hit the same HBM page and halve
        # the gather drain rate)
        order = np.argsort(key, kind="stable")
        self._sorted_idx = idx_local[order]
        self._sorted_dloc = dloc[order]
        ngroups = n_cores * self.nt * N_SLICES
        sizes = np.bincount(key, minlength=ngroups).reshape(
            n_cores, self.nt, N_SLICES
        )
        self._gstart = np.zeros(ngroups + 1, dtype=np.int64)
        np.cumsum(sizes.reshape(-1), out=self._gstart[1:])
        self.sizes = sizes

        nb = (sizes.max(axis=0) + P - 1) // P             # [nt, N_SLICES]
        nb[:, 0] = np.maximum(nb[:, 0], 1)                # >=1 block per tile
        self.nb = nb
        self.bp = np.zeros((N_SLICES, self.nt + 1), dtype=np.int64)
        np.cumsum(nb.T, axis=1, out=self.bp[:, 1:])
        self.blocks_c = self.bp[:, -1].copy()
        self.slots_c = self.blocks_c * P
        self.total_slots = int(self.slots_c.sum())

        # --- supertiles: within slices, capped blocks per chunk -------
        self.supertiles = []
        for j in range(N_SLICES):
            t0 = j * tps
            tend = (j + 1) * tps
            while t0 < tend:
                t1 = t0 + 1
                while t1 < tend and t1 - t0 < MAX_ST_TILES:
                    if any(
                        int(nb[t0:t1 + 1, c].sum()) > CAPC
                        for c in range(N_SLICES)
                    ):
                        break
                    t1 += 1
                self.supertiles.append((t0, t1))
                t0 = t1
        self.max_sg_blocks = max(
            max(int(self.nb[a:b, c].sum()) for c in range(N_SLICES))
            for a, b in self.supertiles
        )
        assert self.max_sg_blocks <= CAPC

        # --- degrees (incl self loop), per core [P, nt] slot order ----
        deg = np.bincount(dst, minlength=n_nodes).astype(np.float32)
        self.degw = np.ones((n_cores, P, self.nt), dtype=np.float32)
        for c in range(n_cores):
            d = np.ones(self.nsh, dtype=np.float32)
            d[self.slot_of[c]] = deg[c * self.base:(c + 1) * self.base]
            self.degw[c] = d.reshape(self.nt, P).T

        # --- per-core gather index / dst_local arrays -----------------
        self.idx16 = []
        self.dstl2 = []
        for c in range(n_cores):
            idx_c, dstl_c = [], []
            for k in range(N_SLICES):
                s = int(self.slots_c[k])
                ia = np.zeros(s, dtype=np.int16)
                da = np.full(s, PAD_DST, dtype=np.float32)
                for t in range(self.nt):
                    g = (c * self.nt + t) * N_SLICES + k
                    a, b = self._gstart[g], self._gstart[g + 1]
                    o = int(self.bp[k][t]) * P
                    n = int(b - a)
                    ia[o:o + n] = self._sorted_idx[a:b].astype(np.int16)
                    da[o:o + n] = self._sorted_dloc[a:b]
                idx_c.append(np.ascontiguousarray(
                    np.tile(ia.reshape(-1, 16).T, (P // 16, 1))
                ).astype(np.int16))
                # [P, blocks*2]: each block's dst_local duplicated in an
                # adjacent pair (keeps the DVE one-hot build 2x-eligible)
                dw = np.ascontiguousarray(da.reshape(-1, P).T)  # [P, blocks]
                dstl_c.append(np.repeat(dw, 2, axis=1).astype(NPBF16))
            self.idx16.append(idx_c)
            self.dstl2.append(dstl_c)


def build_nc(plan: Plan):
    nc = bacc.Bacc(
        "TRN2",
        target_bir_lowering=False,
        debug=False,
        enable_asserts=False,
        num_devices=plan.n_cores,
        num_swdge_queues=N_SLICES,
    )
    fin, hid, fout = plan.fin, plan.hid, plan.fout
    nt, nsh, kch, tps = plan.nt, plan.nsh, plan.kch, plan.tps
    ssz = plan.slice_sz

    xT = nc.dram_tensor("xT", [fin, nsh], BF16, kind="ExternalInput")
    degw = nc.dram_tensor("degw", [P, nt], F32, kind="ExternalInput")
    w1 = nc.dram_tensor("w1", [fin, hid], BF16, kind="ExternalInput")
    b1 = nc.dram_tensor("b1", [1, hid], F32, kind="ExternalInput")
    w2 = nc.dram_tensor("w2", [hid, fout], F32, kind="ExternalInput")
    b2 = nc.dram_tensor("b2", [1, fout], F32, kind="ExternalInput")
    idx_d = [
        nc.dram_tensor(f"idx{c}", [P, int(plan.slots_c[c]) // 16], I16,
                       kind="ExternalInput")
        for c in range(N_SLICES)
    ]
    dstl_d = [
        nc.dram_tensor(f"dstl{c}", [P, 2 * int(plan.blocks_c[c])], BF16,
                       kind="ExternalInput")
        for c in range(N_SLICES)
    ]
    out = nc.dram_tensor("out", [nsh, fout], F32, kind="ExternalOutput")

    table1 = [
        nc.dram_tensor(f"table1_{c}", [plan.ch, TROW], BF16,
                       kind="Internal", addr_space="Shared")
        for c in range(N_SLICES)
    ]
    table2 = [
        nc.dram_tensor(f"table2_{c}", [plan.ch, TROW], BF16,
                       kind="Internal", addr_space="Shared")
        for c in range(N_SLICES)
    ]

    rg = [list(range(plan.n_cores))]

    with tile.TileContext(nc) as tc:
        with (
            tc.tile_pool(name="const", bufs=1) as cp,
            tc.tile_pool(name="dram", bufs=1, space="DRAM") as dp,
        ):
            # ---- constants -------------------------------------------------
            iota = cp.tile([P, P], BF16)
            nc.gpsimd.iota(iota[:], pattern=[[1, P]], base=0,
                           channel_multiplier=0,
                           allow_small_or_imprecise_dtypes=True)
            ident = cp.tile([P, P], F32)
            make_identity(nc, ident[:])

            w1sb = cp.tile([P, kch, hid], BF16)
            nc.sync.dma_start(
                w1sb[:], w1.ap().rearrange("(a p) f -> p a f", p=P)
            )
            w2sb = cp.tile([hid, fout], F32)
            nc.sync.dma_start(w2sb[:], w2.ap())
            b1row = cp.tile([P, hid], F32)
            nc.sync.dma_start(b1row[:], b1.ap().to_broadcast([P, hid]))
            b2row = cp.tile([P, fout], F32)
            nc.sync.dma_start(b2row[:], b2.ap().to_broadcast([P, fout]))

            degt = cp.tile([P, nt], F32)
            nc.sync.dma_start(degt[:], degw.ap())
            rec = cp.tile([P, nt], F32)
            nc.vector.reciprocal(rec[:], degt[:])
            dinv = cp.tile([P, nt], F32)
            nc.scalar.activation(dinv[:], rec[:], AF.Sqrt)

            idxsb = []
            dstlsb = []
            for c in range(N_SLICES):
                it = cp.tile([P, int(plan.slots_c[c]) // 16], I16,
                             tag=f"idx{c}")
                nc.sync.dma_start(it[:], idx_d[c].ap())
                idxsb.append(it)
                dt_ = cp.tile([P, 2 * int(plan.blocks_c[c])], BF16,
                              tag=f"dstl{c}")
                nc.sync.dma_start(dt_[:], dstl_d[c].ap())
                dstlsb.append(dt_)

            m_all = cp.tile([P, nt], F32)
            ssum_all = cp.tile([P, nt], F32)
            out_loc = cp.tile([P, nt, fout], F32)

            ybounce = [dp.tile([ssz, TROW], BF16, name=f"ybounce{c}")
                       for c in range(N_SLICES)]
            zbounce = [dp.tile([ssz, TROW], BF16, name=f"zbounce{c}")
                       for c in range(N_SLICES)]

            # ---- HAM warm-up: ~5us of dummy matmuls ------------------------
            with (
                tc.tile_pool(name="warm", bufs=1) as wp,
                tc.tile_pool(name="warmps", bufs=2, space="PSUM") as wpp,
            ):
                wa = wp.tile([P, P], BF16)
                nc.vector.memset(wa[:], 0.0)
                wa2 = wp.tile([P, 512], BF16)
                nc.vector.memset(wa2[:], 0.0)
                for i in range(12):
                    wps = wpp.tile([P, 512], F32, tag="wps")
                    nc.tensor.matmul(wps[:], lhsT=wa[:], rhs=wa2[:],
                                     start=True, stop=True)

            # ---- phase 1: y = dinv * (x @ W1) -> per-slice AllGather -------
            WB = 8
            with (
                tc.tile_pool(name="xload", bufs=2) as xp,
                tc.tile_pool(name="ps1", bufs=4, space="PSUM") as pp1,
                tc.tile_pool(name="ystage", bufs=2) as yp,
            ):
                xTap = xT.ap().rearrange("(a p) n -> p a n", p=P)
                y_w = None
                for wb in range(0, nt, WB):
                    nwin = min(WB, nt - wb)
                    xt = xp.tile([P, kch, P * WB], BF16, tag="xt")
                    nc.sync.dma_start(
                        xt[:, :, : P * nwin],
                        xTap[:, :, wb * P:(wb + nwin) * P],
                    )
                    for w in range(nwin):
                        t = wb + w
                        tl = t % tps
                        if tl == 0:
                            y_w = yp.tile([P, tps, hid], BF16, tag="yw")
                        ps = pp1.tile([P, hid], F32, tag="ps1")
                        for a in range(kch):
                            nc.tensor.matmul(
                                ps[:],
                                lhsT=xt[:, a, w * P:(w + 1) * P],
                                rhs=w1sb[:, a, :],
                                start=(a == 0),
                                stop=(a == kch - 1),
                            )
                        nc.vector.tensor_scalar(
                            out=y_w[:, tl, :], in0=ps[:],
                            scalar1=dinv[:, t:t + 1], scalar2=None,
                            op0=OP.mult,
                        )
                        if tl == tps - 1:
                            j = t // tps
                            nc.sync.dma_start(
                                ybounce[j][:].rearrange(
                                    "(t p) f -> p t f", p=P)[:, :, :hid],
                                y_w[:],
                            )
                            nc.gpsimd.collective_compute(
                                "AllGather", OP.bypass, replica_groups=rg,
                                ins=[ybounce[j].opt()],
                                outs=[table1[j].ap()],
                            )

            # ---- aggregation pass (both layers) ----------------------------
            def aggregate(tables, epilogue, lname):
                with (
                    tc.tile_pool(name=f"gath{lname}", bufs=4) as gp,
                    tc.tile_pool(name=f"stp{lname}", bufs=4) as stp,
                    tc.tile_pool(name=f"acc{lname}", bufs=2,
                                 space="PSUM") as pacc,
                    tc.tile_pool(name=f"eps{lname}", bufs=3) as ep,
                    tc.tile_pool(name=f"psT{lname}", bufs=2,
                                 space="PSUM") as ppT,
                    tc.tile_pool(name=f"pso{lname}", bufs=2,
                                 space="PSUM") as ppo,
                ):
                    for (t0, t1) in plan.supertiles:
                        nts = t1 - t0
                        acc = pacc.tile([P, MAX_ST_TILES * hid], F32,
                                        tag="acc")
                        done = [0] * nts
                        total = [int(plan.nb[t].sum()) for t in range(t0, t1)]
                        for c in range(N_SLICES):
                            blk0 = int(plan.bp[c][t0])
                            blk1 = int(plan.bp[c][t1])
                            nbg = blk1 - blk0
                            if nbg == 0:
                                continue
                            yb = gp.tile([P, CAPC, TROW], BF16, tag="yb")
                            nc.gpsimd.dma_gather(
                                yb[:, :nbg, :],
                                tables[c].ap(),
                                idxsb[c][:, blk0 * 8:blk1 * 8],
                                nbg * P,
                                nbg * P,
                                TROW,
                                single_packet=False,
                                queue_num=c,
                            )
                            st = stp.tile([P, CAPC, P], BF16, tag="st")
                            nc.vector.tensor_tensor(
                                out=st[:, :nbg, :],
                                in0=iota[:].rearrange(
                                    "p (a d) -> p a d", a=1
                                ).to_broadcast([P, nbg, P]),
                                in1=dstlsb[c][:, blk0 * 2:blk1 * 2].rearrange(
                                    "p (b e) -> p b e", e=2
                                )[:, :, 0:1].to_broadcast([P, nbg, P]),
                                op=OP.is_equal,
                            )
                            for t in range(t0, t1):
                                i = t - t0
                                b0 = int(plan.bp[c][t])
                                b1_ = int(plan.bp[c][t + 1])
                                for b in range(b0, b1_):
                                    nc.tensor.matmul(
                                        acc[:, i * hid:(i + 1) * hid],
                                        lhsT=st[:, b - blk0, :],
                                        rhs=yb[:, b - blk0, :hid],
                                        start=(done[i] == 0),
                                        stop=(done[i] == total[i] - 1),
                                    )
                                    done[i] += 1
                        for t in range(t0, t1):
                            epilogue(t, acc[:, (t - t0) * hid:
                                            (t - t0 + 1) * hid], ep, ppT, ppo)

            # ---- layer-1 epilogue: z = dinv*relu(dinv*acc + b1) ------------
            zstage = {}

            def epi1(t, acc, ep, ppT, ppo):
                tl = t % tps
                j = t // tps
                if tl == 0:
                    zstage[j] = ep.tile([P, tps, hid], BF16, tag="zw",
                                        name=f"zw{j}")
                a2 = ep.tile([P, hid], F32, tag="a2")
                nc.vector.scalar_tensor_tensor(
                    out=a2[:], in0=acc, scalar=dinv[:, t:t + 1],
                    in1=b1row[:], op0=OP.mult, op1=OP.add,
                )
                nc.scalar.activation(
                    zstage[j][:, tl, :], a2[:], AF.Relu,
                    scale=dinv[:, t:t + 1],
                )
                if tl == tps - 1:
                    nc.sync.dma_start(
                        zbounce[j][:].rearrange(
                            "(t p) f -> p t f", p=P)[:, :, :hid],
                        zstage[j][:],
                    )
                    nc.gpsimd.collective_compute(
                        "AllGather", OP.bypass, replica_groups=rg,
                        ins=[zbounce[j].opt()],
                        outs=[table2[j].ap()],
                    )

            aggregate(table1, epi1, "L1")

            # ---- layer-2 epilogue: log_softmax(dinv*acc @ W2 + b2) ---------
            def epi2(t, acc, ep, ppT, ppo):
                u = ep.tile([P, hid], F32, tag="u")
                nc.vector.tensor_scalar(
                    out=u[:], in0=acc, scalar1=dinv[:, t:t + 1],
                    scalar2=None, op0=OP.mult,
                )
                pT = ppT.tile([hid, P], F32, tag="pT")
                nc.tensor.transpose(out=pT[:], in_=u[:], identity=ident[:])
                uT = ep.tile([hid, P], F32, tag="uT")
                nc.scalar.copy(uT[:], pT[:])
                po = ppo.tile([P, fout], F32, tag="po")
                nc.tensor.matmul(po[:], lhsT=uT[:], rhs=w2sb[:],
                                 start=True, stop=True)
                nc.vector.tensor_tensor(
                    out=out_loc[:, t, :], in0=po[:], in1=b2row[:], op=OP.add
                )
                nc.vector.reduce_max(m_all[:, t:t + 1], out_loc[:, t, :],
                                     axis=mybir.AxisListType.X, negate=True)
                e = ep.tile([P, fout], F32, tag="e")
                nc.scalar.activation(e[:], out_loc[:, t, :], AF.Exp,
                                     bias=m_all[:, t:t + 1],
                                     accum_out=ssum_all[:, t:t + 1])

            aggregate(table2, epi2, "L2")

            # deferred log-sum-exp: out -= log(ssum) - m_all (m_all = -max)
            lse_all = cp.tile([P, nt], F32)
            nc.scalar.activation(lse_all[:], ssum_all[:], AF.Ln)
            c_all = cp.tile([P, nt], F32)
            nc.vector.tensor_tensor(
                out=c_all[:], in0=lse_all[:], in1=m_all[:], op=OP.subtract
            )
            for t in range(nt):
                nc.vector.tensor_scalar(
                    out=out_loc[:, t, :], in0=out_loc[:, t, :],
                    scalar1=c_all[:, t:t + 1], scalar2=None,
                    op0=OP.subtract,
                )
            nc.sync.dma_start(
                out.ap().rearrange("(t p) f -> p t f", p=P), out_loc[:]
            )

    nc.compile()
    return nc


def make_in_maps(plan: Plan, x, W1, b1, W2, b2):
    x = np.asarray(x, dtype=np.float32)
    w1b = np.ascontiguousarray(W1, dtype=np.float32).astype(NPBF16)
    in_maps = []
    for c in range(plan.n_cores):
        xT = np.zeros((plan.fin, plan.nsh), dtype=NPBF16)
        xs = x[c * plan.base:(c + 1) * plan.base, :].astype(NPBF16)
        xT[:, plan.slot_of[c]] = xs.T
        m = {
            "xT": xT,
            "degw": plan.degw[c],
            "w1": w1b,
            "b1": np.asarray(b1, dtype=np.float32).reshape(1, -1),
            "w2": np.ascontiguousarray(W2, dtype=np.float32),
            "b2": np.asarray(b2, dtype=np.float32).reshape(1, -1),
        }
        for k in range(N_SLICES):
            m[f"idx{k}"] = plan.idx16[c][k]
            m[f"dstl{k}"] = plan.dstl2[c][k]
        in_maps.append(m)
    return in_maps


_CACHE = {}


def _get_compiled(n_nodes, fin, hid, fout, edge_key, edge_index):
    key = (n_nodes, fin, hid, fout, edge_key)
    if key not in _CACHE:
        plan = Plan(n_nodes, fin, hid, fout, edge_index)
        nc = build_nc(plan)
        _CACHE[key] = (plan, nc)
    return _CACHE[key]


def kernel(x, edge_index, W1, b1, W2, b2, _trace=False):
    x = np.asarray(x)
    edge_index = np.asarray(edge_index)
    n_nodes, fin = x.shape
    hid = np.asarray(W1).shape[1]
    fout = np.asarray(W2).shape[1]
    edge_key = hash(edge_index.tobytes())
    plan, nc = _get_compiled(n_nodes, fin, hid, fout, edge_key, edge_index)
    in_maps = make_in_maps(plan, x, W1, b1, W2, b2)
    res = bass_utils.run_bass_kernel_spmd(
        nc, in_maps, core_ids=list(range(plan.n_cores)), trace=_trace
    )
    parts = [
        res.results[c]["out"][plan.slot_of[c], :]
        for c in range(plan.n_cores)
    ]
    out = np.concatenate(parts, axis=0).astype(np.float32)
    kernel.last_results = res
    return out


# revision 12
# speedup vs baseline: 2.0167x; 1.6039x over previous
"""Two-layer GCN (nn_Method_GCN_11098195493080) as a Bass/Tile kernel on 8
Trainium2 NeuronCores.

Strategy (1D graph partition; dst-owned edges; halo via AllGather):
  - Nodes sharded 8 ways; per core, slots are tile-major (slot = t*128+p)
    and split into 4 row-slices of nt/4 tiles.  The halo exchange is 4
    *pipelined* per-slice AllGathers per layer, each fired as soon as its
    slice's table rows are written, so the collectives overlap compute
    instead of serializing the kernel (the single big AllGather per layer
    stalled every engine for 120-160us).
  - Layer 1: y = dinv * (x_shard @ W1) on PE (bf16), row-major per tile.
  - Aggregation (both layers): batched dma_gather fetches 256B bf16 rows
    edge-major on 4 SWDGE queues (queue = source slice); a one-hot
    ST[128 edges, 128 dst] built on the vector engine turns the segment
    sum into PE matmuls accumulated in PSUM.  Self loops are added
    densely from the local shard (w_loc), saving ~6% of gather traffic.
  - Layer 2 aggregates h *before* applying W2; the @W2 + bias +
    log_softmax runs per-tile with a deferred log-sum-exp fixup.
  - Each (core, slice) permutes its nodes into 26 tiles of 128 slots
    balancing per-(tile, source-slice) edge counts, so nearly every
    group packs into blocks of 128 gathered slots.
  - Host-side work is integer graph partitioning only.
"""

import numpy as np
import ml_dtypes

import concourse.bass as bass
import concourse.bacc as bacc
import concourse.mybir as mybir
import concourse.tile as tile
from concourse import bass_utils
from concourse.masks import make_identity

F32 = mybir.dt.float32
BF16 = mybir.dt.bfloat16
I16 = mybir.dt.int16
AF = mybir.ActivationFunctionType
OP = mybir.AluOpType
NPBF16 = ml_dtypes.bfloat16

N_CORES = 8
N_SLICES = 4          # row-slices per core = chunks = AllGather stages
P = 128               # partitions / dst-tile size
PAD_DST = 254.0       # dst_local value whose one-hot row is all-zero
SLOT_CAP = 6144       # max gathered edge slots per supertile buffer
TROW = 128            # table row width (bf16): hid data + zero pad = 256B


def _balance_slice(d4, ntile, cap_nodes=P):
    """Assign nodes of one (core, slice) to ntile tiles, balancing
    per-(tile, chunk) edge loads.  Returns slot index (t*128+p)."""
    n = d4.shape[0]
    order = np.argsort(-d4.sum(axis=1), kind="stable")
    loads = np.zeros((ntile, N_SLICES), dtype=np.int64)
    counts = np.zeros(ntile, dtype=np.int64)
    slot_of = np.empty(n, dtype=np.int64)
    full_penalty = np.zeros(ntile, dtype=np.int64)
    for node in order:
        d = d4[node]
        score = (loads + d).max(axis=1) + full_penalty
        t = int(np.argmin(score))
        loads[t] += d
        slot_of[node] = t * cap_nodes + counts[t]
        counts[t] += 1
        if counts[t] >= cap_nodes:
            full_penalty[t] = 1 << 40
    return slot_of


class Plan:
    """Static, core-uniform schedule derived from the (integer) graph."""

    def __init__(self, n_nodes, fin, hid, fout, edge_index, n_cores=N_CORES):
        assert n_nodes % n_cores == 0
        self.n_nodes = n_nodes
        self.fin, self.hid, self.fout = fin, hid, fout
        self.n_cores = n_cores
        self.base = n_nodes // n_cores
        tps = (self.base // N_SLICES + P - 1) // P + 1   # tiles per slice
        self.tps = tps
        self.nt = tps * N_SLICES
        self.nsh = self.nt * P
        self.slice_sz = tps * P
        self.ch = self.slice_sz * n_cores
        assert self.ch <= 32767, "chunk must fit int16 gather index"
        self.ntab = self.ch * N_SLICES
        assert fin % P == 0
        self.kch = fin // P
        self.nps = self.base // N_SLICES         # real nodes per slice

        src = np.asarray(edge_index[0], dtype=np.int64)
        dst = np.asarray(edge_index[1], dtype=np.int64)
        s_owner = src // self.base
        d_owner = dst // self.base
        s_local = src - s_owner * self.base
        d_local = dst - d_owner * self.base
        s_slice = np.minimum(s_local // self.nps, N_SLICES - 1)
        d_slice = np.minimum(d_local // self.nps, N_SLICES - 1)

        # --- per (core, slice) balance on per-chunk in-degree ---------
        self.slot_of = np.empty((n_cores, self.base), dtype=np.int64)
        for c in range(n_cores):
            for j in range(N_SLICES):
                lo, hi = j * self.nps, min((j + 1) * self.nps, self.base)
                sel = (d_owner == c) & (d_slice == j)
                dl = d_local[sel] - lo
                ck = s_slice[sel]
                d4 = np.zeros((hi - lo, N_SLICES), dtype=np.int64)
                np.add.at(d4, (dl, ck), 1)
                sl = _balance_slice(d4, tps)
                self.slot_of[c, lo:hi] = j * self.slice_sz + sl

        # --- slot-space edge endpoints (tile-major table rows) --------
        d_slot = self.slot_of[d_owner, d_local]
        s_slot = self.slot_of[s_owner, s_local]
        chunk = s_slot // self.slice_sz
        idx_local = s_owner * self.slice_sz + (s_slot - chunk * self.slice_sz)
        tile_id = d_slot // P
        dloc = (d_slot % P).astype(np.float32)

        key = (d_owner * self.nt + tile_id) * N_SLICES + chunk
        order = np.argsort(key, kind="stable")
        self._sorted_idx = idx_local[order]
        self._sorted_dloc = dloc[order]
        ngroups = n_cores * self.nt * N_SLICES
        sizes = np.bincount(key, minlength=ngroups).reshape(
            n_cores, self.nt, N_SLICES
        )
        self._gstart = np.zeros(ngroups + 1, dtype=np.int64)
        np.cumsum(sizes.reshape(-1), out=self._gstart[1:])
        self.sizes = sizes

        nb = (sizes.max(axis=0) + P - 1) // P             # [nt, N_SLICES]
        nb[:, 0] = np.maximum(nb[:, 0], 1)
        self.nb = nb
        self.bp = np.zeros((N_SLICES, self.nt + 1), dtype=np.int64)
        np.cumsum(nb.T, axis=1, out=self.bp[:, 1:])
        self.blocks_c = self.bp[:, -1].copy()
        self.slots_c = self.blocks_c * P
        self.total_slots = int(self.slots_c.sum())

        # --- supertiles: within slices, capped total slots ------------
        self.supertiles = []
        for j in range(N_SLICES):
            t0 = j * tps
            tend = (j + 1) * tps
            while t0 < tend:
                t1 = t0 + 1
                while t1 < tend:
                    tot = int(nb[t0:t1 + 1].sum()) * P
                    if tot > SLOT_CAP:
                        break
                    t1 += 1
                self.supertiles.append((t0, t1))
                t0 = t1
        self.max_sg_blocks = max(
            int(self.nb[a:b].sum()) for a, b in self.supertiles
        )

        # degrees (with self loop), per core wrapped [128, nt] slot order
        deg = np.bincount(dst, minlength=n_nodes).astype(np.float32) + 1.0
        self.degw = np.ones((n_cores, P, self.nt), dtype=np.float32)
        for c in range(n_cores):
            d = np.ones(self.nsh, dtype=np.float32)
            d[self.slot_of[c]] = deg[c * self.base:(c + 1) * self.base]
            self.degw[c] = d.reshape(self.nt, P).T

        # per-core gather index / dst_local arrays in slot order
        self.idx16 = []
        self.dstl = []
        for c in range(n_cores):
            idx_c, dstl_c = [], []
            for k in range(N_SLICES):
                s = int(self.slots_c[k])
                ia = np.zeros(s, dtype=np.int16)
                da = np.full(s, PAD_DST, dtype=np.float32)
                for t in range(self.nt):
                    g = (c * self.nt + t) * N_SLICES + k
                    a, b = self._gstart[g], self._gstart[g + 1]
                    o = int(self.bp[k][t]) * P
                    n = int(b - a)
                    ia[o:o + n] = self._sorted_idx[a:b].astype(np.int16)
                    da[o:o + n] = self._sorted_dloc[a:b]
                idx_c.append(np.ascontiguousarray(
                    np.tile(ia.reshape(-1, 16).T, (P // 16, 1))
                ).astype(np.int16))
                dstl_c.append(np.ascontiguousarray(da.reshape(-1, P).T))
            self.idx16.append(idx_c)
            self.dstl.append(dstl_c)


def build_nc(plan: Plan):
    nc = bacc.Bacc(
        "TRN2",
        target_bir_lowering=False,
        debug=False,
        enable_asserts=False,
        num_devices=plan.n_cores,
        num_swdge_queues=N_SLICES,
    )
    fin, hid, fout = plan.fin, plan.hid, plan.fout
    nt, nsh, kch, tps = plan.nt, plan.nsh, plan.kch, plan.tps
    ssz = plan.slice_sz

    xT = nc.dram_tensor("xT", [fin, nsh], BF16, kind="ExternalInput")
    degw = nc.dram_tensor("degw", [P, nt], F32, kind="ExternalInput")
    w1 = nc.dram_tensor("w1", [fin, hid], BF16, kind="ExternalInput")
    b1 = nc.dram_tensor("b1", [1, hid], F32, kind="ExternalInput")
    w2 = nc.dram_tensor("w2", [hid, fout], F32, kind="ExternalInput")
    b2 = nc.dram_tensor("b2", [1, fout], F32, kind="ExternalInput")
    idx_d = [
        nc.dram_tensor(f"idx{c}", [P, int(plan.slots_c[c]) // 16], I16,
                       kind="ExternalInput")
        for c in range(N_SLICES)
    ]
    dstl_d = [
        nc.dram_tensor(f"dstl{c}", [P, int(plan.blocks_c[c])], F32,
                       kind="ExternalInput")
        for c in range(N_SLICES)
    ]
    out = nc.dram_tensor("out", [nsh, fout], F32, kind="ExternalOutput")

    table1 = [
        nc.dram_tensor(f"table1_{c}", [plan.ch, TROW], BF16,
                       kind="Internal", addr_space="Shared")
        for c in range(N_SLICES)
    ]
    table2 = [
        nc.dram_tensor(f"table2_{c}", [plan.ch, TROW], BF16,
                       kind="Internal", addr_space="Shared")
        for c in range(N_SLICES)
    ]

    rg = [list(range(plan.n_cores))]

    with tile.TileContext(nc) as tc:
        with (
            tc.tile_pool(name="const", bufs=1) as cp,
            tc.tile_pool(name="dram", bufs=1, space="DRAM") as dp,
        ):
            # ---- constants -------------------------------------------------
            iota = cp.tile([P, P], BF16)
            nc.gpsimd.iota(iota[:], pattern=[[1, P]], base=0,
                           channel_multiplier=0,
                           allow_small_or_imprecise_dtypes=True)
            ident = cp.tile([P, P], F32)
            make_identity(nc, ident[:])

            w1sb = cp.tile([P, kch, hid], BF16)
            nc.sync.dma_start(
                w1sb[:], w1.ap().rearrange("(a p) f -> p a f", p=P)
            )
            w2sb = cp.tile([hid, fout], F32)
            nc.sync.dma_start(w2sb[:], w2.ap())
            b1row = cp.tile([P, hid], F32)
            nc.sync.dma_start(b1row[:], b1.ap().to_broadcast([P, hid]))
            b2row = cp.tile([P, fout], F32)
            nc.sync.dma_start(b2row[:], b2.ap().to_broadcast([P, fout]))

            degt = cp.tile([P, nt], F32)
            nc.sync.dma_start(degt[:], degw.ap())
            rec = cp.tile([P, nt], F32)
            nc.vector.reciprocal(rec[:], degt[:])
            dinv = cp.tile([P, nt], F32)
            nc.scalar.activation(dinv[:], rec[:], AF.Sqrt)

            idxsb = []
            dstlsb = []
            for c in range(N_SLICES):
                it = cp.tile([P, int(plan.slots_c[c]) // 16], I16,
                             tag=f"idx{c}")
                nc.sync.dma_start(it[:], idx_d[c].ap())
                idxsb.append(it)
                dt_ = cp.tile([P, int(plan.blocks_c[c])], F32,
                              tag=f"dstl{c}")
                nc.sync.dma_start(dt_[:], dstl_d[c].ap())
                dstlsb.append(dt_)

            # local table shards (row = TROW bf16: hid data + zero pad)
            w_loc = cp.tile([P, nt, hid], F32)     # dinv*y + b1
            m_all = cp.tile([P, nt], F32)
            ssum_all = cp.tile([P, nt], F32)
            y_loc = cp.tile([P, nt, TROW], BF16)
            z_loc = cp.tile([P, nt, TROW], BF16)
            nc.vector.memset(y_loc[:], 0.0)
            nc.vector.memset(z_loc[:], 0.0)
            out_loc = cp.tile([P, nt, fout], F32)

            ybounce = [dp.tile([ssz, TROW], BF16, name=f"ybounce{c}")
                       for c in range(N_SLICES)]
            zbounce = [dp.tile([ssz, TROW], BF16, name=f"zbounce{c}")
                       for c in range(N_SLICES)]

            def send_slice(j, loc, bounce, table):
                nc.sync.dma_start(
                    bounce[j][:].rearrange("(t p) f -> p t f", p=P),
                    loc[:, j * tps:(j + 1) * tps, :],
                )
                nc.gpsimd.collective_compute(
                    "AllGather", OP.bypass, replica_groups=rg,
                    ins=[bounce[j].opt()],
                    outs=[table[j].ap()],
                )

            # ---- phase 1: y = dinv * (x @ W1) -> per-slice AllGather -------
            WB = 8
            with (
                tc.tile_pool(name="xload", bufs=2) as xp,
                tc.tile_pool(name="ps1", bufs=4, space="PSUM") as pp1,
            ):
                xTap = xT.ap().rearrange("(a p) n -> p a n", p=P)
                for wb in range(0, nt, WB):
                    nwin = min(WB, nt - wb)
                    xt = xp.tile([P, kch, P * WB], BF16, tag="xt")
                    nc.sync.dma_start(
                        xt[:, :, : P * nwin],
                        xTap[:, :, wb * P:(wb + nwin) * P],
                    )
                    for w in range(nwin):
                        t = wb + w
                        ps = pp1.tile([P, hid], F32, tag="ps1")
                        for a in range(kch):
                            nc.tensor.matmul(
                                ps[:],
                                lhsT=xt[:, a, w * P:(w + 1) * P],
                                rhs=w1sb[:, a, :],
                                start=(a == 0),
                                stop=(a == kch - 1),
                            )
                        nc.vector.tensor_scalar(
                            out=y_loc[:, t, :hid], in0=ps[:],
                            scalar1=dinv[:, t:t + 1], scalar2=None,
                            op0=OP.mult,
                        )
                        nc.vector.scalar_tensor_tensor(
                            out=w_loc[:, t, :], in0=y_loc[:, t, :hid],
                            scalar=dinv[:, t:t + 1], in1=b1row[:],
                            op0=OP.mult, op1=OP.add,
                        )
                        if (t + 1) % tps == 0:
                            send_slice(t // tps, y_loc, ybounce, table1)

            # ---- aggregation pass (both layers) ----------------------------
            def aggregate(tables, epilogue, on_slice_done, lname):
                with (
                    tc.tile_pool(name=f"gath{lname}", bufs=2) as gp,
                    tc.tile_pool(name=f"stp{lname}", bufs=4) as stp,
                    tc.tile_pool(name=f"ps2{lname}", bufs=3,
                                 space="PSUM") as pp2,
                    tc.tile_pool(name=f"eps{lname}", bufs=3) as ep,
                    tc.tile_pool(name=f"psT{lname}", bufs=2,
                                 space="PSUM") as ppT,
                    tc.tile_pool(name=f"pso{lname}", bufs=2,
                                 space="PSUM") as ppo,
                ):
                    for (t0, t1) in plan.supertiles:
                        off = {}
                        yb = gp.tile([P, plan.max_sg_blocks, TROW], BF16,
                                     tag="yb")
                        o = 0
                        for c in range(N_SLICES):
                            blk0 = int(plan.bp[c][t0])
                            blk1 = int(plan.bp[c][t1])
                            nbg = blk1 - blk0
                            off[c] = (o, blk0)
                            if nbg == 0:
                                continue
                            nc.gpsimd.dma_gather(
                                yb[:, o:o + nbg, :],
                                tables[c].ap(),
                                idxsb[c][:, blk0 * 8:blk1 * 8],
                                nbg * P,
                                nbg * P,
                                TROW,
                                single_packet=False,
                                queue_num=c,
                            )
                            o += nbg
                        for t in range(t0, t1):
                            ps = pp2.tile([P, hid], F32, tag="ps2")
                            total = int(plan.nb[t].sum())
                            sts = {}
                            for c in range(N_SLICES):
                                nbt = int(plan.nb[t][c])
                                if nbt == 0:
                                    continue
                                b0 = int(plan.bp[c][t])
                                st = stp.tile([P, nbt, P], BF16, tag="st",
                                              name=f"st{c}")
                                nc.vector.tensor_tensor(
                                    out=st[:],
                                    in0=iota[:].rearrange(
                                        "p (a f) -> p a f", a=1
                                    ).to_broadcast([P, nbt, P]),
                                    in1=dstlsb[c][:, b0:b0 + nbt].rearrange(
                                        "p (b o) -> p b o", o=1
                                    ).to_broadcast([P, nbt, P]),
                                    op=OP.is_equal,
                                )
                                sts[c] = st
                            done = 0
                            for c in range(N_SLICES):
                                o, blk0 = off[c]
                                for b in range(int(plan.bp[c][t]),
                                               int(plan.bp[c][t + 1])):
                                    nc.tensor.matmul(
                                        ps[:],
                                        lhsT=sts[c][:, b - int(plan.bp[c][t]),
                                                    :],
                                        rhs=yb[:, o + (b - blk0), :hid],
                                        start=(done == 0),
                                        stop=(done == total - 1),
                                    )
                                    done += 1
                            epilogue(t, ps, ep, ppT, ppo)
                        if (t1 % tps) == 0 and on_slice_done is not None:
                            on_slice_done(t1 // tps - 1)

            # ---- layer-1 epilogue: z = dinv*relu(dinv*(s+y) + b1) ----------
            def epi1(t, ps, ep, ppT, ppo):
                a2 = ep.tile([P, hid], F32, tag="a2")
                nc.vector.scalar_tensor_tensor(
                    out=a2[:], in0=ps[:], scalar=dinv[:, t:t + 1],
                    in1=w_loc[:, t, :], op0=OP.mult, op1=OP.add,
                )
                nc.scalar.activation(
                    z_loc[:, t, :hid], a2[:], AF.Relu,
                    scale=dinv[:, t:t + 1],
                )

            def zslice_done(j):
                send_slice(j, z_loc, zbounce, table2)

            aggregate(table1, epi1, zslice_done, "L1")

            # ---- layer-2 epilogue: log_softmax(dinv*(s+z) @ W2 + b2) -------
            def epi2(t, ps, ep, ppT, ppo):
                u = ep.tile([P, hid], F32, tag="u")
                nc.vector.tensor_tensor(
                    out=u[:], in0=ps[:], in1=z_loc[:, t, :hid], op=OP.add
                )
                opre = ep.tile([P, hid], F32, tag="a1")
                nc.scalar.mul(opre[:], u[:], dinv[:, t:t + 1])
                pT = ppT.tile([hid, P], F32, tag="pT")
                nc.tensor.transpose(out=pT[:], in_=opre[:],
                                    identity=ident[:])
                opT = ep.tile([hid, P], F32, tag="opT")
                nc.scalar.copy(opT[:], pT[:])
                po = ppo.tile([P, fout], F32, tag="po")
                nc.tensor.matmul(po[:], lhsT=opT[:], rhs=w2sb[:],
                                 start=True, stop=True)
                nc.vector.tensor_tensor(
                    out=out_loc[:, t, :], in0=po[:], in1=b2row[:], op=OP.add
                )
                nc.vector.reduce_max(m_all[:, t:t + 1], out_loc[:, t, :],
                                     axis=mybir.AxisListType.X, negate=True)
                e = ep.tile([P, fout], F32, tag="e")
                nc.scalar.activation(e[:], out_loc[:, t, :], AF.Exp,
                                     bias=m_all[:, t:t + 1],
                                     accum_out=ssum_all[:, t:t + 1])

            aggregate(table2, epi2, None, "L2")

            # deferred log-sum-exp: out -= log(ssum) - m_all (m_all = -max)
            lse_all = cp.tile([P, nt], F32)
            nc.scalar.activation(lse_all[:], ssum_all[:], AF.Ln)
            c_all = cp.tile([P, nt], F32)
            nc.vector.tensor_tensor(
                out=c_all[:], in0=lse_all[:], in1=m_all[:], op=OP.subtract
            )
            for t in range(nt):
                nc.vector.tensor_scalar(
                    out=out_loc[:, t, :], in0=out_loc[:, t, :],
                    scalar1=c_all[:, t:t + 1], scalar2=None,
                    op0=OP.subtract,
                )
            nc.sync.dma_start(
                out.ap().rearrange("(t p) f -> p t f", p=P), out_loc[:]
            )

    nc.compile()
    return nc


def make_in_maps(plan: Plan, x, W1, b1, W2, b2):
    x = np.asarray(x, dtype=np.float32)
    w1b = np.ascontiguousarray(W1, dtype=np.float32).astype(NPBF16)
    in_maps = []
    for c in range(plan.n_cores):
        xT = np.zeros((plan.fin, plan.nsh), dtype=NPBF16)
        xs = x[c * plan.base:(c + 1) * plan.base, :].astype(NPBF16)
        xT[:, plan.slot_of[c]] = xs.T
        m = {
            "xT": xT,
            "degw": plan.degw[c],
            "w1": w1b,
            "b1": np.asarray(b1, dtype=np.float32).reshape(1, -1),
            "w2": np.ascontiguousarray(W2, dtype=np.float32),
            "b2": np.asarray(b2, dtype=np.float32).reshape(1, -1),
        }
        for k in range(N_SLICES):
            m[f"idx{k}"] = plan.idx16[c][k]
            m[f"dstl{k}"] = plan.dstl[c][k]
        in_maps.append(m)
    return in_maps


_CACHE = {}


def _get_compiled(n_nodes, fin, hid, fout, edge_key, edge_index):
    key = (n_nodes, fin, hid, fout, edge_key)
    if key not in _CACHE:
        plan = Plan(n_nodes, fin, hid, fout, edge_index)
        nc = build_nc(plan)
        _CACHE[key] = (plan, nc)
    return _CACHE[key]


def kernel(x, edge_index, W1, b1, W2, b2, _trace=False):
    x = np.asarray(x)
    edge_index = np.asarray(edge_index)
    n_nodes, fin = x.shape
    hid = np.asarray(W1).shape[1]
    fout = np.asarray(W2).shape[1]
    edge_key = hash(edge_index.tobytes())
    plan, nc = _get_compiled(n_nodes, fin, hid, fout, edge_key, edge_index)
    in_maps = make_in_maps(plan, x, W1, b1, W2, b2)
    res = bass_utils.run_bass_kernel_spmd(
        nc, in_maps, core_ids=list(range(plan.n_cores)), trace=_trace
    )
    parts = [
        res.results[c]["out"][plan.slot_of[c], :]
        for c in range(plan.n_cores)
    ]
    out = np.concatenate(parts, axis=0).astype(np.float32)
    kernel.last_results = res
    return out
